# revision 1
# baseline (speedup 1.0000x reference)
"""Chamfer distance loss kernel for Trainium2 (8 NeuronCores, SPMD).

Problem: B=4 batches, N=M=8192 points, D=3.  loss = sum over batches of
  sum_i min_j ||c1_i - c2_j||^2  +  sum_j min_i ||c2_j - c1_i||^2

Sharding: the 4 batches x 2 directions give exactly 8 independent
(A-cloud, B-cloud) brute-force nearest-neighbor tasks - one per core.
No collectives needed.

Shipped design (build_nc2): direct (a-b)^2 on ScalarE + VectorE.
B-side coords are replicated across all 128 partitions once (a single
stride-0 broadcast DMA); then each 128-point row-tile of A costs six
big [128, 8192] instructions:
  ACT Square(xB + (-xA)) / Square(yB + (-yA)) / Square(zB + (-zA))
  (per-partition bias = that partition's A coordinate), two DVE adds,
  and one DVE min-reduce -> mins[:, t].  The [128, 64] per-point minima
  go back to the host, which does the final (tiny) sum in float64.
Exact fp32 distance math, no |a|^2+|b|^2-2ab cancellation -- measured
4e-8 relative error vs the fp32 reference.

An alternative TensorE implementation (build_nc: K=24 exact-bf16-split
feature matmul producing the full distance matrix in PSUM + DVE
min-reduce) is kept below for reference; it is numerically equally good
but uses ~6x more instructions, which dominates cost in this axon
environment (per-instruction overhead >> cost-model time).

Toolchain notes: walrus here accepts at most ONE sync-wait command per
instruction (and none on custom ISA ops), while Tile emits fused
multi-waits -- _split_waits() hoists extras into standalone event-
semaphore instructions.  _strip_self_waits() removes same-engine waits
(guaranteed by program order) since each semaphore wait costs ~10us in
this environment.
"""

import numpy as np

try:
    import concourse.bass as bass  # noqa: F401
except ImportError:  # harness may run with a bare sys.path
    import sys

    for p in ("/root/.axon_site/_ro/trn_rl_repo", "/opt/trn_rl_repo", "/opt/pypackages"):
        if p not in sys.path:
            sys.path.append(p)
    import concourse.bass as bass  # noqa: F401

import ml_dtypes

B, N, M, D = 4, 8192, 8192, 3
KFEAT = 24
NCORES = 8
PT = 128          # A points per row-tile (PSUM partitions)
BLK = 512         # B points per matmul (one fp32 PSUM bank)
GROUP_BLKS = 4    # matmul banks per vector reduce ([128, 2048])

_BF16 = ml_dtypes.bfloat16


def _split3(v):
    """Exact 3-way bf16 split of fp32: v == vh + vl + vll (8+8+8 mantissa)."""
    vh = v.astype(_BF16).astype(np.float32)
    r = v - vh
    vl = r.astype(_BF16).astype(np.float32)
    vll = (r - vl).astype(_BF16).astype(np.float32)
    return vh, vl, vll


def _features(A, Bc):
    """Build the K=24 augmented feature matrices.

    A: [n,3] row-side cloud, Bc: [m,3] column-side cloud.
    Returns FA [24,n] bf16, FB [24,m] bf16 with
      FA[:,i] . FB[:,j] ~= ||A_i - B_j||^2  (fp32-accurate)
    """
    A = np.asarray(A, np.float32)
    Bc = np.asarray(Bc, np.float32)
    sqA = (A * A).sum(-1, dtype=np.float32)
    sqB = (Bc * Bc).sum(-1, dtype=np.float32)
    FA, FB = [], []
    for k in range(3):
        ah, al, all_ = _split3(A[:, k])
        bh, bl, bll = _split3(Bc[:, k])
        # kept products: hh, hl, lh, ll, h*ll, ll*h  (each exact in fp32)
        FA += [ah, ah, al, al, ah, all_]
        FB += [-2 * bh, -2 * bl, -2 * bh, -2 * bl, -2 * bll, -2 * bh]
    a1, a2, a3 = _split3(sqA)
    ones_m = np.ones_like(sqB)
    FA += [a1, a2, a3]
    FB += [ones_m, ones_m, ones_m]
    b1, b2, b3 = _split3(sqB)
    ones_n = np.ones_like(sqA)
    FA += [ones_n, ones_n, ones_n]
    FB += [b1, b2, b3]
    fa = np.stack(FA, 0).astype(_BF16)
    fb = np.stack(FB, 0).astype(_BF16)
    return fa, fb


import re as _re

_SELF_WAIT_RE = _re.compile(r"^(Pool|Activation|PE|DVE|SP)(_sequencer)?_\d+$")


def _strip_self_waits(nc):
    """Remove semaphore waits where an instruction waits on its OWN engine's
    proc semaphore.  Engines execute their instruction streams in order with
    in-order data completion (DVE/ACT drain between ops; PE matmul ends are
    pc-monotone), so these waits are redundant — and sem waits are extremely
    expensive (~10us) in this environment.  Cross-engine and DMA-proc waits
    are kept."""
    for f in nc.m.functions:
        for bb in f.blocks:
            for ins in bb.instructions:
                si = ins.sync_info
                if not si or not si.on_wait:
                    continue
                eng = str(ins.engine.value) if hasattr(ins.engine, "value") else str(ins.engine)
                kept = []
                for w in si.on_wait:
                    m = _SELF_WAIT_RE.match(w.ant_name or "")
                    if m and m.group(1) == eng:
                        continue
                    kept.append(w)
                if len(kept) != len(si.on_wait):
                    ins.sync_info = mybir_mod().SyncInfo(
                        on_wait=kept, on_update=list(si.on_update)
                    )
    return nc


def mybir_mod():
    from concourse import mybir

    return mybir


def _split_waits(nc, max_waits=1):
    """Walrus in this toolchain accepts at most one sync-wait command per
    instruction; Tile fuses several.  Hoist extra waits into standalone
    event-semaphore instructions right before the owner (same engine, so
    program order preserves semantics)."""
    from concourse import mybir

    for f in nc.m.functions:
        for bb in f.blocks:
            new_insts = []
            for ins in bb.instructions:
                si = ins.sync_info
                waits = list(si.on_wait) if si and si.on_wait else []
                # custom bass_isa instructions (e.g. PartitionBroadcast) can't
                # carry sync waits through walrus's visitInstISA at all
                lim = 0 if "bass_isa" in type(ins).__module__ else max_waits
                if len(waits) > lim:
                    extra, keep = (waits, []) if lim == 0 else (waits[:-lim], waits[-lim:])
                    for k, w in enumerate(extra):
                        ev = mybir.InstEventSemaphore(
                            name=f"{ins.name}-evw{k}", ins=[], outs=[]
                        )
                        ev.engine = ins.engine
                        ev.sync_info = mybir.SyncInfo(on_wait=[w], on_update=[])
                        new_insts.append(ev)
                    ins.sync_info = mybir.SyncInfo(
                        on_wait=keep, on_update=list(si.on_update)
                    )
                new_insts.append(ins)
            bb.instructions[:] = new_insts
    return nc


def build_nc(n_a=N, n_b=M, reps=1, group_blks=GROUP_BLKS, psum_bufs=2):
    """Build the per-core Bass program (SPMD: same program, per-core data)."""
    import concourse.tile as tile
    from concourse import mybir

    row_tiles = n_a // PT
    nblk = n_b // BLK
    ngroups = nblk // group_blks
    assert n_a % PT == 0 and n_b % (BLK * group_blks) == 0

    nc = bass.Bass("TRN2", target_bir_lowering=False, debug=False, num_devices=NCORES)
    # one packed input tensor -> a single input DMA (keeps the kernel-tail
    # drain within walrus's sync-wait-command limit)
    feat_d = nc.dram_tensor(
        "feat", [KFEAT, n_a + n_b], mybir.dt.bfloat16, kind="ExternalInput"
    )
    out_d = nc.dram_tensor("out", [PT, 1], mybir.dt.float32, kind="ExternalOutput")

    with tile.TileContext(nc) as tc:
        with (
            tc.tile_pool(name="const", bufs=1) as cpool,
            tc.tile_pool(name="psum", bufs=psum_bufs, space="PSUM") as ppool,
            tc.tile_pool(name="accum", bufs=1) as apool,
        ):
            feat = cpool.tile([KFEAT, n_a + n_b], mybir.dt.bfloat16)
            nc.sync.dma_start(feat[:], feat_d[:])
            af = feat[:, :n_a]
            bf = feat[:, n_a:]

            mins = apool.tile([PT, row_tiles * ngroups], mybir.dt.float32)
            m2 = apool.tile([PT, row_tiles], mybir.dt.float32)
            res = apool.tile([PT, 1], mybir.dt.float32)

            for _ in range(reps):
                for t in range(row_tiles):
                    lhsT = af[:, t * PT:(t + 1) * PT]
                    for g in range(ngroups):
                        ps = ppool.tile([PT, BLK * group_blks], mybir.dt.float32)
                        for q in range(group_blks):
                            j = g * group_blks + q
                            nc.tensor.matmul(
                                ps[:, q * BLK:(q + 1) * BLK],
                                lhsT,
                                bf[:, j * BLK:(j + 1) * BLK],
                                start=True,
                                stop=True,
                            )
                        nc.vector.tensor_reduce(
                            mins[:, t * ngroups + g: t * ngroups + g + 1],
                            ps[:],
                            axis=mybir.AxisListType.X,
                            op=mybir.AluOpType.min,
                        )
                nc.vector.tensor_reduce(
                    m2[:],
                    mins[:].rearrange("p (t g) -> p t g", g=ngroups),
                    axis=mybir.AxisListType.X,
                    op=mybir.AluOpType.min,
                )
                nc.vector.tensor_reduce(
                    res[:],
                    m2[:],
                    axis=mybir.AxisListType.X,
                    op=mybir.AluOpType.add,
                )
            nc.sync.dma_start(out_d[:], res[:])
    return _split_waits(_strip_self_waits(nc))


def build_nc2(n_a=N, n_b=M, reps=1, post=True):
    """v2: direct (a-b)^2 on ScalarE/VectorE, no PE/PSUM.

    Per 128-point row-tile of A (6 instructions):
      ACT: t1 = Square(xB_rep + (-xA))   [128, n_b]
      ACT: t2 = Square(yB_rep + (-yA))
      DVE: t1 += t2
      ACT: t2 = Square(zB_rep + (-zA))
      DVE: t1 += t2
      DVE: mins[:, t] = reduce_min(t1)
    B coords are replicated across partitions once via gpsimd
    partition_broadcast.  Exact fp32 distance math (no cancellation).
    """
    import concourse.tile as tile
    from concourse import mybir

    row_tiles = n_a // PT
    assert n_a % PT == 0

    nc = bass.Bass("TRN2", target_bir_lowering=False, debug=False, num_devices=NCORES)
    bc_d = nc.dram_tensor("bc", [1, 3 * n_b], mybir.dt.float32, kind="ExternalInput")
    ac_d = nc.dram_tensor("ac", [PT, 3 * row_tiles], mybir.dt.float32, kind="ExternalInput")
    out_d = nc.dram_tensor("out", [PT, row_tiles], mybir.dt.float32, kind="ExternalOutput")

    with tile.TileContext(nc) as tc:
        with tc.tile_pool(name="rep", bufs=1) as rpool:
            rep = rpool.tile([PT, 3 * n_b], mybir.dt.float32)
            # replicate the B coords across all 128 partitions in one DMA
            # (stride-0 partition dim on the DRAM side)
            nc.sync.dma_start(rep[:], bc_d[:].partition_broadcast(PT))
            with tc.tile_pool(name="work", bufs=1) as wpool:
                ac = wpool.tile([PT, 3 * row_tiles], mybir.dt.float32)
                nc.sync.dma_start(ac[:], ac_d[:])
                mins = wpool.tile([PT, row_tiles], mybir.dt.float32)
                t1 = wpool.tile([PT, n_b], mybir.dt.float32)
                t2 = wpool.tile([PT, n_b], mybir.dt.float32)
                xr = rep[:, 0:n_b]
                yr = rep[:, n_b:2 * n_b]
                zr = rep[:, 2 * n_b:3 * n_b]
                SQ = mybir.ActivationFunctionType.Square
                for _ in range(reps):
                    for t in range(row_tiles):
                        nxa = ac[:, t:t + 1]
                        nya = ac[:, row_tiles + t:row_tiles + t + 1]
                        nza = ac[:, 2 * row_tiles + t:2 * row_tiles + t + 1]
                        nc.scalar.activation(t1[:], xr, SQ, bias=nxa)
                        nc.scalar.activation(t2[:], yr, SQ, bias=nya)
                        nc.vector.tensor_tensor(t1[:], t1[:], t2[:], op=mybir.AluOpType.add)
                        nc.scalar.activation(t2[:], zr, SQ, bias=nza)
                        nc.vector.tensor_tensor(t1[:], t1[:], t2[:], op=mybir.AluOpType.add)
                        nc.vector.tensor_reduce(
                            mins[:, t:t + 1], t1[:],
                            axis=mybir.AxisListType.X, op=mybir.AluOpType.min,
                        )
                nc.sync.dma_start(out_d[:], mins[:])
    if post:
        return _split_waits(_strip_self_waits(nc))
    return nc


def make_in_maps2(cloud1, cloud2):
    """v2 inputs: bc = [xB||yB||zB] fp32, ac = negated A coords per row-tile."""
    in_maps = []
    for b in range(B):
        for A, Bc in ((cloud1[b], cloud2[b]), (cloud2[b], cloud1[b])):
            A = np.asarray(A, np.float32)
            Bc = np.asarray(Bc, np.float32)
            n_a, n_b = A.shape[0], Bc.shape[0]
            rt = n_a // PT
            bc = Bc.T.reshape(1, 3 * n_b).astype(np.float32)
            ac = np.concatenate(
                [-A[:, k].reshape(rt, PT).T for k in range(3)], axis=1
            ).astype(np.float32)
            in_maps.append({"bc": np.ascontiguousarray(bc), "ac": np.ascontiguousarray(ac)})
    return in_maps


def make_in_maps(cloud1, cloud2):
    """Per-core inputs: core 2b+0 handles (c1[b]->c2[b]), 2b+1 the reverse."""
    in_maps = []
    for b in range(B):
        for A, Bc in ((cloud1[b], cloud2[b]), (cloud2[b], cloud1[b])):
            fa, fb = _features(A, Bc)
            in_maps.append({"feat": np.concatenate([fa, fb], axis=1)})
    return in_maps


_NC_CACHE = {}


def kernel(cloud1, cloud2):
    from concourse.bass_utils import run_bass_kernel_spmd

    cloud1 = np.asarray(cloud1, np.float32)
    cloud2 = np.asarray(cloud2, np.float32)
    assert cloud1.shape == (B, N, D) and cloud2.shape == (B, M, D)

    if "nc2" not in _NC_CACHE:
        _NC_CACHE["nc2"] = build_nc2()
    nc = _NC_CACHE["nc2"]

    in_maps = make_in_maps2(cloud1, cloud2)
    results = run_bass_kernel_spmd(nc, in_maps, list(range(NCORES))).results
    total = 0.0
    for c in range(NCORES):
        total += float(results[c]["out"].astype(np.float64).sum())
    return np.array(total, dtype=np.float32)



# revision 2
# speedup vs baseline: 14.7613x; 14.7613x over previous
"""Chamfer distance loss kernel for Trainium2 (8 NeuronCores, SPMD) — v6.

Problem: B=4 batches, N=M=8192 points, D=3.  loss = sum over batches of
  sum_i min_j ||c1_i - c2_j||^2  +  sum_j min_i ||c2_j - c1_i||^2

Sharding: the 4 batches x 2 directions give exactly 8 independent
(A-cloud, B-cloud) brute-force nearest-neighbor tasks - one per core.
No collectives needed.

Design (measured, not guessed - this axon environment charges ~40-75us
per *straight-line* instruction per call and input upload at ~450MB/s,
while on-device per-instruction dispatch is a few us and element work
almost free):

 1. A hardware For_i loop over row-tile groups keeps the program tiny
    (~200 static instructions) while executing the full 64-row-tile
    sweep; straight-line unrolls of the same work measure 10-16x slower
    per body in this environment.
 2. Within a group of G=3 row tiles the DVE work is batched into big
    instructions: two [128, 3*8192] fp16 adds and one [128, 3, 8192]
    grouped min-reduce, i.e. ~4.3 instructions per row tile (6 in the
    previously shipped kernel, all straight-line).
 3. unroll=2 groups per loop iteration measured fastest (11 iterations;
    fewer per-iteration barrier resets than unroll=1, less serial body
    than unroll=11).

Per iteration: 1 DVE rolling bias copy + 2 x (9 ACT Square(b + (-a)) +
2 DVE adds + 1 DVE grouped min-reduce) = 27 instructions for 6 row
tiles.

Toolchain workarounds (all discovered by experiment in this axon env):
 - ACT's *bias* operand ignores dynamic (loop-variable) AP offsets - the
   offset register never updates, so every iteration would reuse tile
   0's bias.  DVE dynamic offsets (reads and writes) do work; hence the
   per-iteration "rolling copy" of the bias scalars into a fixed SBUF
   slot that the ACT instructions address statically.
 - walrus accepts at most ONE sync-wait command per instruction and NONE
   on ISA-extended instructions ("ISA wrong length"); _split_waits()
   hoists extras into standalone event-semaphore instructions.
   (This also rules out InstTensorTensorReduce and the custom-DVE ops -
   this walrus build rejects their encodings outright.)
 - Same-engine proc-semaphore waits are redundant (engines execute their
   streams in order) and expensive here; _strip_self_waits() drops them.
 - Sequential or nested For_i bodies fail to lower ("min() arg is an
   empty sequence"), so the timing-rep knob widens the on-chip bias ring
   and the loop bound instead: identical static code, the device loops
   `reps` times over the same work.  Inputs/outputs do NOT scale with
   reps (the bias ring is replicated on-device by small DMAs and only
   the first rep's minima are downloaded), so a reps-delta isolates pure
   on-device time.  (Verified: with per-rep distinct bias data every
   rep's output is correct, so the loop really executes reps full
   sweeps.)

Numerics: squares are computed exactly in fp32 inside ACT
(Square(b + (-a)) - no |a|^2+|b|^2-2ab cancellation), rounded to fp16
for the adds/min-reduce.  fp16's 2^-11 relative error applies to the
*distance itself*, so the near-minimum distances that reach the loss
carry ~5e-4 relative error with random sign; measured total loss error
is ~8e-7 vs the fp32 reference (gate is 2e-2).
"""

import numpy as np

try:
    import concourse.bass as bass  # noqa: F401
except ImportError:  # harness may run with a bare sys.path
    import sys

    for p in ("/root/.axon_site/_ro/trn_rl_repo", "/opt/trn_rl_repo", "/opt/pypackages"):
        if p not in sys.path:
            sys.path.append(p)
    import concourse.bass as bass  # noqa: F401

import re as _re

import bass_rust as _br

B, N, M, D = 4, 8192, 8192, 3
NCORES = 8
PT = 128            # A points per row tile (SBUF partitions)
RT = N // PT        # 64 row tiles
G = 3               # row tiles per group (one grouped DVE add/reduce set)
VT = 66             # virtual tiles (64 real + 2 dummies -> 22 full groups)
NG = VT // G        # 22 groups
GW = 3 * G          # bias columns per group (x,y,z interleaved per tile)
UNROLL = 2          # groups per hardware-loop iteration (must divide NG)

_SELF_WAIT_RE = _re.compile(r"^(Pool|Activation|PE|DVE|SP)(_sequencer)?_\d+$")


def _strip_self_waits(nc):
    """Remove semaphore waits where an instruction waits on its OWN engine's
    proc semaphore.  Engines execute their instruction streams in order with
    in-order data completion, so these waits are redundant - and sem waits
    are expensive in this environment.  Cross-engine and DMA waits kept."""
    from concourse import mybir

    for f in nc.m.functions:
        for bb in f.blocks:
            for ins in bb.instructions:
                si = ins.sync_info
                if not si or not si.on_wait:
                    continue
                eng = str(ins.engine.value) if hasattr(ins.engine, "value") else str(ins.engine)
                kept = []
                for w in si.on_wait:
                    m = _SELF_WAIT_RE.match(w.ant_name or "")
                    if m and m.group(1) == eng:
                        continue
                    kept.append(w)
                if len(kept) != len(si.on_wait):
                    ins.sync_info = mybir.SyncInfo(
                        on_wait=kept, on_update=list(si.on_update)
                    )
    return nc


def _split_waits(nc, max_waits=1):
    """walrus accepts at most one sync-wait command per instruction (and none
    on ISA-extended instructions); Tile emits fused multi-waits.  Hoist the
    extras into standalone event-semaphore instructions right before the
    owner (same engine, so program order preserves semantics)."""
    from concourse import mybir

    for f in nc.m.functions:
        for bb in f.blocks:
            new_insts = []
            for ins in bb.instructions:
                si = ins.sync_info
                waits = list(si.on_wait) if si and si.on_wait else []
                lim = 0 if isinstance(ins, _br.InstISA) else max_waits
                if len(waits) > lim:
                    extra, keep = (waits, []) if lim == 0 else (waits[:-lim], waits[-lim:])
                    for k, w in enumerate(extra):
                        ev = mybir.InstEventSemaphore(
                            name=f"{ins.name}-evw{k}", ins=[], outs=[]
                        )
                        ev.engine = ins.engine
                        ev.sync_info = mybir.SyncInfo(on_wait=[w], on_update=[])
                        new_insts.append(ev)
                    ins.sync_info = mybir.SyncInfo(
                        on_wait=keep, on_update=list(si.on_update)
                    )
                new_insts.append(ins)
            bb.instructions[:] = new_insts
    return nc


def build_nc(reps=1, unroll=UNROLL, post=True):
    """Build the per-core Bass program (SPMD: same program, per-core data).

    Inputs (per core):
      bc  [1, 3*M]     fp32  B-side coords [x||y||z]; replicated across the
                             128 partitions on-chip by one broadcast DMA.
      aci [PT, GW*NG]  fp32  negated A coords, interleaved per group:
                             column 9*g + 3*u + k = -coord_k of row tile
                             t=3g+u (tiles >= RT are dummies), partition =
                             point-within-tile.
    Output:
      out [PT, GW*NG]  fp16  per-point min squared distances; group g's
                             three tiles land in columns 9*g + {0,1,2}.

    `reps` repeats the whole sweep on-device (identical static code, wider
    bias ring + loop bound) for loop-delta timing; I/O does not scale.
    """
    import concourse.tile as tile
    from concourse import mybir as mb
    from concourse.bass import ds

    n_b = M
    W1 = GW * NG
    W = W1 * reps
    assert NG % unroll == 0
    nc = bass.Bass("TRN2", target_bir_lowering=False, debug=False, num_devices=NCORES)
    bc_d = nc.dram_tensor("bc", [1, 3 * n_b], mb.dt.float32, kind="ExternalInput")
    aci_d = nc.dram_tensor("aci", [PT, W1], mb.dt.float32, kind="ExternalInput")
    out_d = nc.dram_tensor("out", [PT, W1], mb.dt.float16, kind="ExternalOutput")
    SQ = mb.ActivationFunctionType.Square
    ADD = mb.AluOpType.add
    MIN = mb.AluOpType.min

    with tile.TileContext(nc) as tc:
        with tc.tile_pool(name="rep", bufs=1) as rpool:
            rep = rpool.tile([PT, 3 * n_b], mb.dt.float32)
            nc.sync.dma_start(rep[:], bc_d[:].partition_broadcast(PT))
            with tc.tile_pool(name="work", bufs=1) as wpool:
                aci = wpool.tile([PT, W], mb.dt.float32)
                for r in range(reps):  # on-device bias-ring replication
                    nc.sync.dma_start(aci[:, r * W1:(r + 1) * W1], aci_d[:])
                mins = wpool.tile([PT, W], mb.dt.float16)
                cur = wpool.tile([PT, GW * unroll], mb.dt.float32)
                S1 = wpool.tile([PT, G * n_b], mb.dt.float16)
                S2 = wpool.tile([PT, G * n_b], mb.dt.float16)
                xr = rep[:, 0:n_b]
                yr = rep[:, n_b:2 * n_b]
                zr = rep[:, 2 * n_b:3 * n_b]
                UW = GW * unroll
                with tc.For_i(0, W, UW) as o:
                    # rolling copy: this iteration's bias scalars -> fixed slot
                    nc.vector.tensor_scalar(
                        cur[:], aci[:, ds(o, UW)], 1.0, None, op0=mb.AluOpType.mult
                    )
                    for v in range(unroll):
                        cb = v * GW
                        for u in range(G):
                            nc.scalar.activation(
                                S1[:, u * n_b:(u + 1) * n_b], xr, SQ,
                                bias=cur[:, cb + 3 * u:cb + 3 * u + 1])
                            nc.scalar.activation(
                                S2[:, u * n_b:(u + 1) * n_b], yr, SQ,
                                bias=cur[:, cb + 3 * u + 1:cb + 3 * u + 2])
                        nc.vector.tensor_tensor(S1[:], S1[:], S2[:], op=ADD)
                        for u in range(G):
                            nc.scalar.activation(
                                S2[:, u * n_b:(u + 1) * n_b], zr, SQ,
                                bias=cur[:, cb + 3 * u + 2:cb + 3 * u + 3])
                        nc.vector.tensor_tensor(S1[:], S1[:], S2[:], op=ADD)
                        mo = mins[:, ds(o, G)] if v == 0 else mins[:, ds(o + cb, G)]
                        nc.vector.tensor_reduce(
                            mo, S1[:].rearrange("p (g n) -> p g n", g=G),
                            axis=mb.AxisListType.X, op=MIN)
                nc.sync.dma_start(out_d[:], mins[:, 0:W1])
    if post:
        return _split_waits(_strip_self_waits(nc))
    return nc


def make_in_maps(cloud1, cloud2):
    """Per-core inputs: core 2b+0 handles (c1[b] -> c2[b]), 2b+1 the reverse."""
    in_maps = []
    for b in range(B):
        for A, Bc in ((cloud1[b], cloud2[b]), (cloud2[b], cloud1[b])):
            A = np.asarray(A, np.float32)
            Bc = np.asarray(Bc, np.float32)
            bc = Bc.T.reshape(1, 3 * M).astype(np.float32)
            negA = np.stack([-A[:, k].reshape(RT, PT) for k in range(3)], axis=0)
            cols = np.empty((VT * 3, PT), np.float32)
            for t in range(VT):
                src = t if t < RT else 0
                for k in range(3):
                    cols[3 * t + k] = negA[k][src]
            in_maps.append({
                "bc": np.ascontiguousarray(bc),
                "aci": np.ascontiguousarray(cols.T),
            })
    return in_maps


def extract_loss(results):
    """Sum the per-point minima from all cores' padded fp16 outputs."""
    total = 0.0
    for c in range(NCORES):
        out = np.asarray(results[c]["out"])  # [PT, GW*NG]
        vals = np.stack(
            [out[:, 9 * g + u] for g in range(NG) for u in range(G)
             if 3 * g + u < RT],
            axis=1,
        )
        total += float(vals.astype(np.float64).sum())
    return total


_NC_CACHE = {}


def kernel(cloud1, cloud2):
    from concourse.bass_utils import run_bass_kernel_spmd

    cloud1 = np.asarray(cloud1, np.float32)
    cloud2 = np.asarray(cloud2, np.float32)
    assert cloud1.shape == (B, N, D) and cloud2.shape == (B, M, D)

    if "nc" not in _NC_CACHE:
        _NC_CACHE["nc"] = build_nc()
    nc = _NC_CACHE["nc"]

    in_maps = make_in_maps(cloud1, cloud2)
    results = run_bass_kernel_spmd(nc, in_maps, list(range(NCORES))).results
    return np.array(extract_loss(results), dtype=np.float32)
